# revision 10
# baseline (speedup 1.0000x reference)
# Trainium2 Bass kernel for nn_DSG_STGCN (PLV adjacency + Gumbel graph aug +
# lead-field/DCT projection). Self-contained: hardcodes shapes/sharding.
#
# Math (what the reference actually returns — the 2x GCN + GRU are dead code):
#   s_low[b]   = dct_m @ lead @ z[b].T            -> reassociated:  W_low.T @ zT
#   s_recon[b] = dct_m.T @ s_low[b]               -> (L.T @ G_slice).T @ zT,
#                with G = dct_m.T @ dct_m (input-independent constant)
#   a_aug      = sigmoid((log(e)-log(1-e)+g)/tau), e = .5*p + .5*a,
#                a from PLV threshold (Hilbert phases -> unit phasors -> grams),
#                p = sigmoid(hg @ hg.T), hg = relu((a @ mean_b z) @ w_gae + b)
#
# Sharding (8 cores): voxel-slice (256 rows each) of s_recon over all 64
# batches; k-slice (64 rows) of s_low; PLV grams data-parallel over batch
# (8 batches/core, realized by rotating z per core so the SPMD program is
# identical) with one small AllReduce of [R | P | sum_b z].
import os
import sys
import threading

import numpy as np

sys.path.insert(0, "/opt/trn_rl_repo")

NUM_ELEC = 128
T = 128
V = 2052
H = 64
K = 512
B = 64
VP = 2176  # V padded to 17*128
NCORES = 8
THRESH2 = float((0.5 * B * T) ** 2)  # plv>=0.5 on |.|^2 * (B*T)^2 scale

# Matmul dtype for the output-producing (lead-field/DCT) chain.
# float32r = reduced-precision fp32 matmul mode, 4x faster at N>=256.
USE_F32R = os.environ.get("KERNEL_F32", "0") != "1"

_lock = threading.Lock()
_cache = {}


def _dct_matrix_f64(N, Kd):
    n = np.arange(N)[None, :]
    k = np.arange(Kd)[:, None]
    m = np.sqrt(2.0 / N) * np.cos(np.pi * (2 * n + 1) * k / (2 * N))
    m[0, :] = 1.0 / np.sqrt(N)
    return m


def _consts():
    if "consts" in _cache:
        return _cache["consts"]
    dct = _dct_matrix_f64(V, K)  # [K, V]
    G = (dct.T @ dct).astype(np.float32)  # [V, V]
    dctT = dct.T.astype(np.float32)  # [V, K]
    idx = np.arange(T)
    hf = np.where(idx == 0, 1.0, np.where(idx < T // 2, 2.0, np.where(idx == T // 2, 1.0, 0.0)))
    A = np.fft.ifft(hf[:, None] * np.fft.fft(np.eye(T), axis=0), axis=0)
    Hm = np.imag(A).astype(np.float32)  # [T, T]; Re(analytic) == z
    ident = np.eye(128, dtype=np.float32)
    _cache["consts"] = (G, dctT, Hm, ident)
    return _cache["consts"]


def _build_nc():
    if "nc" in _cache:
        return _cache["nc"]
    import concourse.bacc as bacc
    import concourse.bass as bass
    import concourse.mybir as mybir
    import concourse.tile as tile
    from concourse.mybir import ActivationFunctionType as AF

    f32 = mybir.dt.float32
    f32r = mybir.dt.float32r

    fmm = f32r if USE_F32R else f32

    nc = bacc.Bacc(
        "TRN2",
        target_bir_lowering=False,
        debug=False,
        num_devices=NCORES,
    )

    z_in = nc.dram_tensor("z", [B, 128, T], f32, kind="ExternalInput")
    lead_in = nc.dram_tensor("leadp", [VP, 128], fmm, kind="ExternalInput")
    gsl_in = nc.dram_tensor("gslp", [VP, 256], fmm, kind="ExternalInput")
    wk_in = nc.dram_tensor("wkp", [VP, 68], fmm, kind="ExternalInput")
    hmT_in = nc.dram_tensor("hmT", [128, 128], fmm, kind="ExternalInput")
    id_in = nc.dram_tensor("ident", [128, 128], f32, kind="ExternalInput")
    wgae_in = nc.dram_tensor("wgae", [128, H], f32, kind="ExternalInput")
    bgae_in = nc.dram_tensor("bgae", [H, 1], f32, kind="ExternalInput")
    gum_in = nc.dram_tensor("gum", [128, 128], f32, kind="ExternalInput")

    slow_o = nc.dram_tensor("slow_o", [B, 64, 128], f32, kind="ExternalOutput")
    srec_o = nc.dram_tensor("srec_o", [B, 256, 128], f32, kind="ExternalOutput")
    srem_o = nc.dram_tensor("srem_o", [B, 4, 128], f32, kind="ExternalOutput")
    aaug_o = nc.dram_tensor("aaug_o", [128, 128], f32, kind="ExternalOutput")
    DEBUG = os.environ.get("KERNEL_DEBUG", "0") == "1"
    if DEBUG:
        pre_o = nc.dram_tensor("pre_o", [128, 384], f32, kind="ExternalOutput")
        post_o = nc.dram_tensor("post_o", [128, 384], f32, kind="ExternalOutput")

    with tile.TileContext(nc) as tc:
        with (
            tc.tile_pool(name="cpool", bufs=1) as cpool,
            tc.tile_pool(name="zlpool", bufs=3) as zlpool,
            tc.tile_pool(name="tpool", bufs=2) as tpool,
            tc.tile_pool(name="stpool", bufs=3) as stpool,
            tc.tile_pool(name="psum", bufs=1, space="PSUM") as psum,
            tc.tile_pool(name="dram", bufs=1, space="DRAM") as dram,
        ):
            # ---- constant / weight loads ----
            hm_sb = cpool.tile([128, 128], fmm)
            nc.sync.dma_start(hm_sb[:], hmT_in[:])
            id_sb = cpool.tile([128, 128], f32)
            nc.sync.dma_start(id_sb[:], id_in[:])
            wgae_sb = cpool.tile([128, H], f32)
            nc.sync.dma_start(wgae_sb[:], wgae_in[:])
            bgae_sb = cpool.tile([H, 1], f32)
            nc.sync.dma_start(bgae_sb[:], bgae_in[:])
            gum_sb = cpool.tile([128, 128], f32)
            nc.sync.dma_start(gum_sb[:], gum_in[:])

            lead_sb = cpool.tile([128, 17 * 128], fmm)
            nc.sync.dma_start(lead_sb[:].rearrange("p (c t) -> p c t", c=17), lead_in.rearrange("(c p) t -> p c t", p=128))
            gsl_sb = cpool.tile([128, 17 * 256], fmm)
            nc.sync.dma_start(gsl_sb[:].rearrange("p (c n) -> p c n", c=17), gsl_in.rearrange("(c p) n -> p c n", p=128))
            wk_sb = cpool.tile([128, 17 * 68], fmm)
            nc.sync.dma_start(wk_sb[:].rearrange("p (c n) -> p c n", c=17), wk_in.rearrange("(c p) n -> p c n", p=128))

            # ---- precompute: M2T slice [t,256] and combo weights [t,68] ----
            m2t_ps = psum.tile([128, 256], f32, tag="acc")
            wc_ps = psum.tile([128, 68], f32, tag="wc")
            for k in range(17):
                lk = lead_sb[:, 128 * k : 128 * (k + 1)]
                nc.tensor.matmul(
                    m2t_ps[:],
                    lk,
                    gsl_sb[:, 256 * k : 256 * (k + 1)],
                    start=(k == 0),
                    stop=(k == 16),
                )
            for k in range(17):
                lk = lead_sb[:, 128 * k : 128 * (k + 1)]
                nc.tensor.matmul(
                    wc_ps[:],
                    lk,
                    wk_sb[:, 68 * k : 68 * (k + 1)],
                    start=(k == 0),
                    stop=(k == 16),
                )
            m2t_sb = cpool.tile([128, 256], fmm)
            nc.vector.tensor_copy(m2t_sb[:], m2t_ps[:])
            wc_sb = cpool.tile([128, 68], fmm)
            nc.vector.tensor_copy(wc_sb[:], wc_ps[:])

            # ---- z load + transpose to zT [t, (b e)] ----
            zT_sb = cpool.tile([128, B * 128], fmm)
            x_acc = cpool.tile([128, 128], f32)
            for g in range(8):
                zl = zlpool.tile([128, 8 * 128], f32, tag="zl")
                nc.sync.dma_start(
                    zl[:].rearrange("e (b t) -> e b t", b=8), z_in[8 * g : 8 * (g + 1)].rearrange("b e t -> e b t")
                )
                for bl in range(8):
                    b = 8 * g + bl
                    pt = psum.tile([128, 128], f32, tag="tp", bufs=2)
                    nc.tensor.transpose(pt[:], zl[:, 128 * bl : 128 * (bl + 1)], id_sb[:])
                    nc.vector.tensor_copy(zT_sb[:, 128 * b : 128 * (b + 1)], pt[:])
                if g == 0:
                    # x partial = sum of this core's 8 PLV batches, [e, t] layout
                    nc.vector.tensor_copy(x_acc[:], zl[:, 0:128])
                    for bl in range(1, 8):
                        nc.vector.tensor_add(
                            x_acc[:], x_acc[:], zl[:, 128 * bl : 128 * (bl + 1)]
                        )

            # ---- Hilbert + unit phasors on this core's 8 batches ----
            C_sb = cpool.tile([128, 1024], f32)
            S_sb = cpool.tile([128, 1024], f32)
            for h in range(2):
                rer = zT_sb[:, 512 * h : 512 * (h + 1)]
                re = rer.bitcast(f32)
                hb = psum.tile([128, 512], f32, tag="mm", bufs=2)
                nc.tensor.matmul(hb[:], hm_sb[:], rer)
                imsq = tpool.tile([128, 512], f32, tag="tmpa")
                nc.scalar.activation(imsq[:], hb[:], AF.Square)
                resq = tpool.tile([128, 512], f32, tag="tmpb")
                nc.vector.tensor_mul(resq[:], re, re)
                mag2 = tpool.tile([128, 512], f32, tag="tmpc")
                nc.vector.tensor_add(mag2[:], imsq[:], resq[:])
                mag = tpool.tile([128, 512], f32, tag="tmpd")
                nc.scalar.activation(mag[:], mag2[:], AF.Sqrt)
                rinv = tpool.tile([128, 512], f32, tag="tmpe")
                nc.vector.reciprocal(rinv[:], mag[:])
                nc.vector.tensor_mul(C_sb[:, 512 * h : 512 * (h + 1)], re, rinv[:])
                nc.vector.tensor_mul(S_sb[:, 512 * h : 512 * (h + 1)], hb[:], rinv[:])

            # ---- PLV grams: R = sum_b Cb'Cb + Sb'Sb ; P = sum_b Sb'Cb ----
            r_ps = psum.tile([128, 128], f32, tag="r")
            p_ps = psum.tile([128, 128], f32, tag="p")
            for b in range(8):
                cb = C_sb[:, 128 * b : 128 * (b + 1)]
                nc.tensor.matmul(r_ps[:], cb, cb, start=(b == 0), stop=False)
            for b in range(8):
                sb_ = S_sb[:, 128 * b : 128 * (b + 1)]
                nc.tensor.matmul(r_ps[:], sb_, sb_, start=False, stop=(b == 7))
            for b in range(8):
                nc.tensor.matmul(
                    p_ps[:],
                    S_sb[:, 128 * b : 128 * (b + 1)],
                    C_sb[:, 128 * b : 128 * (b + 1)],
                    start=(b == 0),
                    stop=(b == 7),
                )

            # ---- AllReduce [R | P | x] across the 8 cores ----
            ccin_sb = cpool.tile([128, 384], f32)
            nc.vector.tensor_copy(ccin_sb[:, 0:128], r_ps[:])
            nc.vector.tensor_copy(ccin_sb[:, 128:256], p_ps[:])
            nc.vector.tensor_copy(ccin_sb[:, 256:384], x_acc[:])
            cc_in = dram.tile([128, 384], f32)
            cc_out = dram.tile([128, 384], f32, addr_space="Shared")
            nc.gpsimd.dma_start(cc_in[:], ccin_sb[:])
            nc.gpsimd.collective_compute(
                "AllReduce",
                mybir.AluOpType.add,
                replica_groups=[list(range(NCORES))],
                ins=[cc_in.opt()],
                outs=[cc_out.opt()],
            )
            ccout_sb = cpool.tile([128, 384], f32)
            nc.gpsimd.dma_start(ccout_sb[:], cc_out[:])
            if DEBUG:
                nc.sync.dma_start(pre_o[:], ccin_sb[:])
                nc.sync.dma_start(post_o[:], ccout_sb[:])

            # ---- mains: s_recon voxel slice [256, (b e)] ----
            for v in range(2):
                lhs = m2t_sb[:, 128 * v : 128 * (v + 1)]
                for q in range(4):
                    st = stpool.tile([128, 2048], f32, tag="st")
                    for j in range(4):
                        n = 4 * q + j
                        mm = psum.tile([128, 512], f32, tag="mm", bufs=2)
                        nc.tensor.matmul(
                            mm[:], lhs, zT_sb[:, 512 * n : 512 * (n + 1)]
                        )
                        nc.vector.tensor_copy(st[:, 512 * j : 512 * (j + 1)], mm[:])
                    nc.sync.dma_start(
                        srec_o[16 * q : 16 * (q + 1), 128 * v : 128 * (v + 1), :].rearrange(
                            "b v e -> v b e"
                        ),
                        st[:].rearrange("v (b e) -> v b e", b=16),
                    )

            # ---- mains: combo = [s_low k-slice (64) | s_recon remainder (4)] ----
            for q in range(4):
                cst = stpool.tile([68, 2048], f32, tag="cst", bufs=2)
                for j in range(4):
                    n = 4 * q + j
                    cm = psum.tile([68, 512], f32, tag="mm", bufs=2)
                    nc.tensor.matmul(
                        cm[:], wc_sb[:], zT_sb[:, 512 * n : 512 * (n + 1)]
                    )
                    nc.vector.tensor_copy(cst[:, 512 * j : 512 * (j + 1)], cm[:])
                nc.sync.dma_start(
                    slow_o[16 * q : 16 * (q + 1), :, :].rearrange("b k e -> k b e"),
                    cst[0:64, :].rearrange("k (b e) -> k b e", b=16),
                )
                nc.sync.dma_start(
                    srem_o[16 * q : 16 * (q + 1), :, :].rearrange("b r e -> r b e"),
                    cst[64:68, :].rearrange("r (b e) -> r b e", b=16),
                )

            # ---- epilogue: a_aug (identical on every core) ----
            Rf = ccout_sb[:, 0:128]
            Pf = ccout_sb[:, 128:256]
            xs = ccout_sb[:, 256:384]
            x_sb = cpool.tile([128, 128], f32)
            nc.vector.tensor_scalar_mul(x_sb[:], xs, 1.0 / B)
            pt_ps = psum.tile([128, 128], f32, tag="tp", bufs=2)
            nc.tensor.transpose(pt_ps[:], Pf, id_sb[:])
            i_sb = cpool.tile([128, 128], f32)
            nc.vector.tensor_sub(i_sb[:], Pf, pt_ps[:])
            i2_sb = cpool.tile([128, 128], f32)
            nc.vector.tensor_mul(i2_sb[:], i_sb[:], i_sb[:])
            r2_sb = cpool.tile([128, 128], f32)
            nc.vector.tensor_mul(r2_sb[:], Rf, Rf)
            m2_sb = cpool.tile([128, 128], f32)
            nc.vector.tensor_add(m2_sb[:], i2_sb[:], r2_sb[:])
            a_sb = cpool.tile([128, 128], f32)
            nc.vector.tensor_scalar(
                a_sb[:], m2_sb[:], THRESH2, None, op0=mybir.AluOpType.is_ge
            )
            axT_ps = psum.tile([128, 128], f32, tag="tp", bufs=2)
            nc.tensor.matmul(axT_ps[:], x_sb[:], a_sb[:])
            axT_sb = cpool.tile([128, 128], f32)
            nc.vector.tensor_copy(axT_sb[:], axT_ps[:])
            hg_ps = psum.tile([H, 128], f32, tag="acc")
            nc.tensor.matmul(hg_ps[:], wgae_sb[:], axT_sb[:])
            hg_sb = cpool.tile([H, 128], f32)
            nc.scalar.activation(
                hg_sb[:], hg_ps[:], AF.Relu, bias=bgae_sb[:, 0:1]
            )
            pp_ps = psum.tile([128, 128], f32, tag="r")
            nc.tensor.matmul(pp_ps[:], hg_sb[:], hg_sb[:])
            p_sb = cpool.tile([128, 128], f32)
            nc.scalar.activation(p_sb[:], pp_ps[:], AF.Sigmoid)
            epre_sb = cpool.tile([128, 128], f32)
            nc.vector.tensor_add(epre_sb[:], p_sb[:], a_sb[:])
            le_sb = cpool.tile([128, 128], f32)
            nc.scalar.activation(le_sb[:], epre_sb[:], AF.Ln, scale=0.5)
            l1me_sb = cpool.tile([128, 128], f32)
            nc.scalar.activation(l1me_sb[:], epre_sb[:], AF.Ln, bias=1.0, scale=-0.5)
            lu_sb = cpool.tile([128, 128], f32)
            nc.scalar.activation(lu_sb[:], gum_sb[:], AF.Ln)
            lv_sb = cpool.tile([128, 128], f32)
            nc.scalar.activation(lv_sb[:], lu_sb[:], AF.Ln, scale=-1.0)
            d1_sb = cpool.tile([128, 128], f32)
            nc.vector.tensor_sub(d1_sb[:], le_sb[:], l1me_sb[:])
            d2_sb = cpool.tile([128, 128], f32)
            nc.vector.tensor_sub(d2_sb[:], d1_sb[:], lv_sb[:])
            aaug_sb = cpool.tile([128, 128], f32)
            nc.scalar.activation(aaug_sb[:], d2_sb[:], AF.Sigmoid, scale=10.0)
            nc.sync.dma_start(aaug_o[:], aaug_sb[:])

    nc.compile()
    _cache["nc"] = nc
    return nc


def kernel(z, lead_field, gumbel_u, w_gae, b_gae, **_unused):
    from concourse.bass_utils import run_bass_kernel_spmd

    z = np.ascontiguousarray(np.asarray(z, dtype=np.float32))
    L = np.asarray(lead_field, dtype=np.float32)
    u = np.ascontiguousarray(np.asarray(gumbel_u, dtype=np.float32))
    w_gae = np.ascontiguousarray(np.asarray(w_gae, dtype=np.float32))
    b_gae = np.asarray(b_gae, dtype=np.float32).reshape(H, 1)

    G, dctT, Hm, ident = _consts()
    Lp = np.zeros((VP, 128), np.float32)
    Lp[:V] = L
    hmT = np.ascontiguousarray(Hm.T)

    nc = _build_nc()

    in_maps = []
    for c in range(NCORES):
        gsl = np.zeros((VP, 256), np.float32)
        gsl[:V] = G[:, 256 * c : 256 * (c + 1)]
        wk = np.zeros((VP, 68), np.float32)
        wk[:V, :64] = dctT[:, 64 * c : 64 * (c + 1)]
        wk[:V, 64:] = G[:, 2048:2052]
        in_maps.append(
            {
                "z": np.ascontiguousarray(np.roll(z, -8 * c, axis=0)),
                "leadp": Lp,
                "gslp": gsl,
                "wkp": wk,
                "hmT": hmT,
                "ident": ident,
                "wgae": w_gae,
                "bgae": np.ascontiguousarray(b_gae),
                "gum": u,
            }
        )

    trace = os.environ.get("KERNEL_TRACE", "0") == "1"
    with _lock:
        res = run_bass_kernel_spmd(
            nc, in_maps, core_ids=list(range(NCORES)), trace=trace
        )
    _cache["last_res"] = res
    results = res.results

    s_low = np.empty((B, K, 128), np.float32)
    s_recon = np.empty((B, V, 128), np.float32)
    for c in range(NCORES):
        r = results[c]
        s_low[:, 64 * c : 64 * (c + 1), :] = np.roll(r["slow_o"], 8 * c, axis=0)
        s_recon[:, 256 * c : 256 * (c + 1), :] = np.roll(r["srec_o"], 8 * c, axis=0)
    s_recon[:, 2048:2052, :] = results[0]["srem_o"]
    a_aug = results[0]["aaug_o"]
    return s_low, s_recon, a_aug


# revision 13
# speedup vs baseline: 1.0084x; 1.0084x over previous
# Trainium2 Bass kernel for nn_DSG_STGCN (PLV adjacency + Gumbel graph aug +
# lead-field/DCT projection). Self-contained: hardcodes shapes/sharding.
#
# Math (what the reference actually returns — the 2x GCN + GRU are dead code):
#   s_low[b]   = dct_m @ lead @ z[b].T            -> reassociated:  W_low.T @ zT
#   s_recon[b] = dct_m.T @ s_low[b]               -> (L.T @ G_slice).T @ zT,
#                with G = dct_m.T @ dct_m (input-independent constant)
#   a_aug      = sigmoid((log(e)-log(1-e)+g)/tau), e = .5*p + .5*a,
#                a from PLV threshold (Hilbert phases -> unit phasors -> grams),
#                p = sigmoid(hg @ hg.T), hg = relu((a @ mean_b z) @ w_gae + b)
#
# Sharding (8 cores): voxel-slice (256 rows each) of s_recon over all 64
# batches; k-slice (64 rows) of s_low; PLV grams data-parallel over batch
# (8 batches/core, realized by rotating z per core so the SPMD program is
# identical) with one small AllReduce of [R | P | sum_b z].
import os
import sys
import threading

import numpy as np

sys.path.insert(0, "/opt/trn_rl_repo")

NUM_ELEC = 128
T = 128
V = 2052
H = 64
K = 512
B = 64
VP = 2176  # V padded to 17*128
NCORES = 8
THRESH2 = float((0.5 * B * T) ** 2)  # plv>=0.5 on |.|^2 * (B*T)^2 scale

# Matmul dtype for the output-producing (lead-field/DCT) chain.
# float32r = reduced-precision fp32 matmul mode, 4x faster at N>=256.
USE_F32R = os.environ.get("KERNEL_F32", "0") != "1"

_lock = threading.Lock()
_cache = {}


def _dct_matrix_f64(N, Kd):
    n = np.arange(N)[None, :]
    k = np.arange(Kd)[:, None]
    m = np.sqrt(2.0 / N) * np.cos(np.pi * (2 * n + 1) * k / (2 * N))
    m[0, :] = 1.0 / np.sqrt(N)
    return m


def _consts():
    if "consts" in _cache:
        return _cache["consts"]
    dct = _dct_matrix_f64(V, K)  # [K, V]
    G = (dct.T @ dct).astype(np.float32)  # [V, V]
    dctT = dct.T.astype(np.float32)  # [V, K]
    idx = np.arange(T)
    hf = np.where(idx == 0, 1.0, np.where(idx < T // 2, 2.0, np.where(idx == T // 2, 1.0, 0.0)))
    A = np.fft.ifft(hf[:, None] * np.fft.fft(np.eye(T), axis=0), axis=0)
    Hm = np.imag(A).astype(np.float32)  # [T, T]; Re(analytic) == z
    ident = np.eye(128, dtype=np.float32)
    _cache["consts"] = (G, dctT, Hm, ident)
    return _cache["consts"]


def _build_nc():
    if "nc" in _cache:
        return _cache["nc"]
    import concourse.bacc as bacc
    import concourse.bass as bass
    import concourse.mybir as mybir
    import concourse.tile as tile
    from concourse.mybir import ActivationFunctionType as AF

    f32 = mybir.dt.float32
    f32r = mybir.dt.float32r

    fmm = f32r if USE_F32R else f32

    nc = bacc.Bacc(
        "TRN2",
        target_bir_lowering=False,
        debug=False,
        num_devices=NCORES,
    )

    z_in = nc.dram_tensor("z", [B, 128, T], fmm, kind="ExternalInput")
    lead_in = nc.dram_tensor("leadp", [VP, 128], fmm, kind="ExternalInput")
    gsl_in = nc.dram_tensor("gslp", [VP, 256], fmm, kind="ExternalInput")
    wk_in = nc.dram_tensor("wkp", [VP, 68], fmm, kind="ExternalInput")
    hmT_in = nc.dram_tensor("hmT", [128, 128], fmm, kind="ExternalInput")
    id_in = nc.dram_tensor("ident", [128, 128], fmm, kind="ExternalInput")
    idf_in = nc.dram_tensor("identf", [128, 128], f32, kind="ExternalInput")
    wgae_in = nc.dram_tensor("wgae", [128, H], f32, kind="ExternalInput")
    bgae_in = nc.dram_tensor("bgae", [H, 1], f32, kind="ExternalInput")
    gum_in = nc.dram_tensor("gum", [128, 128], f32, kind="ExternalInput")

    slow_o = nc.dram_tensor("slow_o", [B, 64, 128], f32, kind="ExternalOutput")
    srec_o = nc.dram_tensor("srec_o", [B, 256, 128], f32, kind="ExternalOutput")
    srem_o = nc.dram_tensor("srem_o", [B, 4, 128], f32, kind="ExternalOutput")
    aaug_o = nc.dram_tensor("aaug_o", [128, 128], f32, kind="ExternalOutput")
    DEBUG = os.environ.get("KERNEL_DEBUG", "0") == "1"
    if DEBUG:
        pre_o = nc.dram_tensor("pre_o", [128, 384], f32, kind="ExternalOutput")
        post_o = nc.dram_tensor("post_o", [128, 384], f32, kind="ExternalOutput")

    with tile.TileContext(nc) as tc:
        with (
            tc.tile_pool(name="cpool", bufs=1) as cpool,
            tc.tile_pool(name="zlpool", bufs=3) as zlpool,
            tc.tile_pool(name="tpool", bufs=2) as tpool,
            tc.tile_pool(name="stpool", bufs=3) as stpool,
            tc.tile_pool(name="psum", bufs=1, space="PSUM") as psum,
            tc.tile_pool(name="dram", bufs=1, space="DRAM") as dram,
        ):
            # ---- small constants ----
            id_sb = cpool.tile([128, 128], fmm)
            nc.sync.dma_start(id_sb[:], id_in[:])
            hm_sb = cpool.tile([128, 128], fmm)
            nc.sync.dma_start(hm_sb[:], hmT_in[:])
            idf_sb = cpool.tile([128, 128], f32)
            nc.sync.dma_start(idf_sb[:], idf_in[:])
            wgae_sb = cpool.tile([128, H], f32)
            nc.sync.dma_start(wgae_sb[:], wgae_in[:])
            bgae_sb = cpool.tile([H, 1], f32)
            nc.sync.dma_start(bgae_sb[:], bgae_in[:])
            gum_sb = cpool.tile([128, 128], f32)
            nc.sync.dma_start(gum_sb[:], gum_in[:])

            zT_sb = cpool.tile([128, B * 128], fmm)
            x_acc = cpool.tile([128, 128], f32)
            zls = []

            def load_group(g):
                zl = zlpool.tile([128, 8 * 128], fmm, tag="zl", name=f"zl{g}")
                nc.sync.dma_start(
                    zl[:].rearrange("e (b t) -> e b t", b=8),
                    z_in[8 * g : 8 * (g + 1)].rearrange("b e t -> e b t"),
                )
                return zl

            def transpose_group(g, zl):
                for bl in range(8):
                    b = 8 * g + bl
                    pt = psum.tile([128, 128], fmm, tag="tp", bufs=2, name=f"pt{b}")
                    nc.tensor.transpose(pt[:], zl[:, 128 * bl : 128 * (bl + 1)], id_sb[:])
                    nc.vector.tensor_copy(zT_sb[:, 128 * b : 128 * (b + 1)], pt[:])

            # ---- group 0: load, transpose, PLV chain ----
            zl0 = load_group(0)
            transpose_group(0, zl0)
            nc.vector.tensor_copy(x_acc[:], zl0[:, 0:128].bitcast(f32))
            for bl in range(1, 8):
                nc.vector.tensor_add(
                    x_acc[:], x_acc[:], zl0[:, 128 * bl : 128 * (bl + 1)].bitcast(f32)
                )

            C_sb = cpool.tile([128, 1024], f32)
            S_sb = cpool.tile([128, 1024], f32)
            for h in range(2):
                rer = zT_sb[:, 512 * h : 512 * (h + 1)]
                re = rer.bitcast(f32)
                hb = psum.tile([128, 512], f32, tag="mm", bufs=2)
                nc.tensor.matmul(hb[:], hm_sb[:], rer)
                imsq = tpool.tile([128, 512], f32, tag="tmpa")
                nc.scalar.activation(imsq[:], hb[:], AF.Square)
                resq = tpool.tile([128, 512], f32, tag="tmpb")
                nc.vector.tensor_mul(resq[:], re, re)
                mag2 = tpool.tile([128, 512], f32, tag="tmpc")
                nc.vector.tensor_add(mag2[:], imsq[:], resq[:])
                mag = tpool.tile([128, 512], f32, tag="tmpd")
                nc.scalar.activation(mag[:], mag2[:], AF.Sqrt)
                rinv = tpool.tile([128, 512], f32, tag="tmpe")
                nc.vector.reciprocal(rinv[:], mag[:])
                nc.vector.tensor_mul(C_sb[:, 512 * h : 512 * (h + 1)], re, rinv[:])
                nc.vector.tensor_mul(S_sb[:, 512 * h : 512 * (h + 1)], hb[:], rinv[:])

            r_ps = psum.tile([128, 128], f32, tag="r")
            p_ps = psum.tile([128, 128], f32, tag="p")
            for b in range(8):
                cb = C_sb[:, 128 * b : 128 * (b + 1)]
                nc.tensor.matmul(r_ps[:], cb, cb, start=(b == 0), stop=False)
            for b in range(8):
                sb_ = S_sb[:, 128 * b : 128 * (b + 1)]
                nc.tensor.matmul(r_ps[:], sb_, sb_, start=False, stop=(b == 7))
            for b in range(8):
                nc.tensor.matmul(
                    p_ps[:],
                    S_sb[:, 128 * b : 128 * (b + 1)],
                    C_sb[:, 128 * b : 128 * (b + 1)],
                    start=(b == 0),
                    stop=(b == 7),
                )

            # ---- AllReduce [R | P | x], issued as early as possible ----
            ccin_sb = cpool.tile([128, 384], f32)
            nc.vector.tensor_copy(ccin_sb[:, 0:128], r_ps[:])
            nc.vector.tensor_copy(ccin_sb[:, 128:256], p_ps[:])
            nc.vector.tensor_copy(ccin_sb[:, 256:384], x_acc[:])
            cc_in = dram.tile([128, 384], f32)
            cc_out = dram.tile([128, 384], f32, addr_space="Shared")
            nc.gpsimd.dma_start(cc_in[:], ccin_sb[:])
            nc.gpsimd.collective_compute(
                "AllReduce",
                mybir.AluOpType.add,
                replica_groups=[list(range(NCORES))],
                ins=[cc_in.opt()],
                outs=[cc_out.opt()],
            )
            ccout_sb = cpool.tile([128, 384], f32)
            nc.gpsimd.dma_start(ccout_sb[:], cc_out[:])
            if DEBUG:
                nc.sync.dma_start(pre_o[:], ccin_sb[:])
                nc.sync.dma_start(post_o[:], ccout_sb[:])

            # ---- weight loads + precompute ----
            lead_sb = cpool.tile([128, 17 * 128], fmm)
            nc.sync.dma_start(
                lead_sb[:].rearrange("p (c t) -> p c t", c=17),
                lead_in.rearrange("(c p) t -> p c t", p=128),
            )
            gsl_sb = cpool.tile([128, 17 * 256], fmm)
            nc.sync.dma_start(
                gsl_sb[:].rearrange("p (c n) -> p c n", c=17),
                gsl_in.rearrange("(c p) n -> p c n", p=128),
            )
            wk_sb = cpool.tile([128, 17 * 68], fmm)
            nc.sync.dma_start(
                wk_sb[:].rearrange("p (c n) -> p c n", c=17),
                wk_in.rearrange("(c p) n -> p c n", p=128),
            )

            m2t_ps = psum.tile([128, 256], f32, tag="acc")
            wc_ps = psum.tile([128, 68], f32, tag="wc")
            for k in range(17):
                lk = lead_sb[:, 128 * k : 128 * (k + 1)]
                nc.tensor.matmul(
                    m2t_ps[:],
                    lk,
                    gsl_sb[:, 256 * k : 256 * (k + 1)],
                    start=(k == 0),
                    stop=(k == 16),
                )
            for k in range(17):
                lk = lead_sb[:, 128 * k : 128 * (k + 1)]
                nc.tensor.matmul(
                    wc_ps[:],
                    lk,
                    wk_sb[:, 68 * k : 68 * (k + 1)],
                    start=(k == 0),
                    stop=(k == 16),
                )
            m2t_sb = cpool.tile([128, 256], fmm)
            nc.vector.tensor_copy(m2t_sb[:], m2t_ps[:])
            wc_sb = cpool.tile([128, 68], fmm)
            nc.vector.tensor_copy(wc_sb[:], wc_ps[:])

            # ---- streamed mains: per 8-batch group, transpose then matmul+store ----
            for g in range(8):
                if g == 0:
                    zl = zl0
                else:
                    zl = load_group(g)
                    transpose_group(g, zl)
                zt_g = zT_sb[:, 1024 * g : 1024 * (g + 1)]
                for v in range(2):
                    st = stpool.tile([128, 1024], f32, tag="st", name=f"st{g}_{v}")
                    for j in range(2):
                        mm = psum.tile([128, 512], f32, tag="mm", bufs=2)
                        nc.tensor.matmul(
                            mm[:],
                            m2t_sb[:, 128 * v : 128 * (v + 1)],
                            zt_g[:, 512 * j : 512 * (j + 1)],
                        )
                        nc.vector.tensor_copy(st[:, 512 * j : 512 * (j + 1)], mm[:])
                    nc.sync.dma_start(
                        srec_o[8 * g : 8 * (g + 1), 128 * v : 128 * (v + 1), :].rearrange(
                            "b v e -> v b e"
                        ),
                        st[:].rearrange("v (b e) -> v b e", b=8),
                    )
                cst = stpool.tile([68, 1024], f32, tag="cst", bufs=2, name=f"cst{g}")
                for j in range(2):
                    cm = psum.tile([68, 512], f32, tag="mm", bufs=2)
                    nc.tensor.matmul(
                        cm[:], wc_sb[:], zt_g[:, 512 * j : 512 * (j + 1)]
                    )
                    nc.scalar.activation(cst[:, 512 * j : 512 * (j + 1)], cm[:], AF.Copy)
                nc.sync.dma_start(
                    slow_o[8 * g : 8 * (g + 1), :, :].rearrange("b k e -> k b e"),
                    cst[0:64, :].rearrange("k (b e) -> k b e", b=8),
                )
                nc.sync.dma_start(
                    srem_o[8 * g : 8 * (g + 1), :, :].rearrange("b r e -> r b e"),
                    cst[64:68, :].rearrange("r (b e) -> r b e", b=8),
                )

            # ---- epilogue: a_aug (identical on every core) ----
            Rf = ccout_sb[:, 0:128]
            Pf = ccout_sb[:, 128:256]
            xs = ccout_sb[:, 256:384]
            x_sb = cpool.tile([128, 128], f32)
            nc.vector.tensor_scalar_mul(x_sb[:], xs, 1.0 / B)
            pt_ps = psum.tile([128, 128], f32, tag="p")
            nc.tensor.transpose(pt_ps[:], Pf, idf_sb[:])
            i_sb = cpool.tile([128, 128], f32)
            nc.vector.tensor_sub(i_sb[:], Pf, pt_ps[:])
            i2_sb = cpool.tile([128, 128], f32)
            nc.vector.tensor_mul(i2_sb[:], i_sb[:], i_sb[:])
            r2_sb = cpool.tile([128, 128], f32)
            nc.vector.tensor_mul(r2_sb[:], Rf, Rf)
            m2_sb = cpool.tile([128, 128], f32)
            nc.vector.tensor_add(m2_sb[:], i2_sb[:], r2_sb[:])
            a_sb = cpool.tile([128, 128], f32)
            nc.vector.tensor_scalar(
                a_sb[:], m2_sb[:], THRESH2, None, op0=mybir.AluOpType.is_ge
            )
            axT_ps = psum.tile([128, 128], f32, tag="r")
            nc.tensor.matmul(axT_ps[:], x_sb[:], a_sb[:])
            axT_sb = cpool.tile([128, 128], f32)
            nc.vector.tensor_copy(axT_sb[:], axT_ps[:])
            hg_ps = psum.tile([H, 128], f32, tag="p")
            nc.tensor.matmul(hg_ps[:], wgae_sb[:], axT_sb[:])
            hg_sb = cpool.tile([H, 128], f32)
            nc.scalar.activation(
                hg_sb[:], hg_ps[:], AF.Relu, bias=bgae_sb[:, 0:1]
            )
            pp_ps = psum.tile([128, 128], f32, tag="r")
            nc.tensor.matmul(pp_ps[:], hg_sb[:], hg_sb[:])
            p_sb = cpool.tile([128, 128], f32)
            nc.scalar.activation(p_sb[:], pp_ps[:], AF.Sigmoid)
            epre_sb = cpool.tile([128, 128], f32)
            nc.vector.tensor_add(epre_sb[:], p_sb[:], a_sb[:])
            le_sb = cpool.tile([128, 128], f32)
            nc.scalar.activation(le_sb[:], epre_sb[:], AF.Ln, scale=0.5)
            l1me_sb = cpool.tile([128, 128], f32)
            nc.scalar.activation(l1me_sb[:], epre_sb[:], AF.Ln, bias=1.0, scale=-0.5)
            lu_sb = cpool.tile([128, 128], f32)
            nc.scalar.activation(lu_sb[:], gum_sb[:], AF.Ln)
            lv_sb = cpool.tile([128, 128], f32)
            nc.scalar.activation(lv_sb[:], lu_sb[:], AF.Ln, scale=-1.0)
            d1_sb = cpool.tile([128, 128], f32)
            nc.vector.tensor_sub(d1_sb[:], le_sb[:], l1me_sb[:])
            d2_sb = cpool.tile([128, 128], f32)
            nc.vector.tensor_sub(d2_sb[:], d1_sb[:], lv_sb[:])
            aaug_sb = cpool.tile([128, 128], f32)
            nc.scalar.activation(aaug_sb[:], d2_sb[:], AF.Sigmoid, scale=10.0)
            nc.sync.dma_start(aaug_o[:], aaug_sb[:])

    nc.compile()
    _cache["nc"] = nc
    return nc


def kernel(z, lead_field, gumbel_u, w_gae, b_gae, **_unused):
    from concourse.bass_utils import run_bass_kernel_spmd

    z = np.ascontiguousarray(np.asarray(z, dtype=np.float32))
    L = np.asarray(lead_field, dtype=np.float32)
    u = np.ascontiguousarray(np.asarray(gumbel_u, dtype=np.float32))
    w_gae = np.ascontiguousarray(np.asarray(w_gae, dtype=np.float32))
    b_gae = np.asarray(b_gae, dtype=np.float32).reshape(H, 1)

    G, dctT, Hm, ident = _consts()
    Lp = np.zeros((VP, 128), np.float32)
    Lp[:V] = L
    hmT = np.ascontiguousarray(Hm.T)

    nc = _build_nc()

    in_maps = []
    for c in range(NCORES):
        gsl = np.zeros((VP, 256), np.float32)
        gsl[:V] = G[:, 256 * c : 256 * (c + 1)]
        wk = np.zeros((VP, 68), np.float32)
        wk[:V, :64] = dctT[:, 64 * c : 64 * (c + 1)]
        wk[:V, 64:] = G[:, 2048:2052]
        in_maps.append(
            {
                "z": np.ascontiguousarray(np.roll(z, -8 * c, axis=0)),
                "leadp": Lp,
                "gslp": gsl,
                "wkp": wk,
                "hmT": hmT,
                "ident": ident,
                "identf": ident,
                "wgae": w_gae,
                "bgae": np.ascontiguousarray(b_gae),
                "gum": u,
            }
        )

    trace = os.environ.get("KERNEL_TRACE", "0") == "1"
    with _lock:
        res = run_bass_kernel_spmd(
            nc, in_maps, core_ids=list(range(NCORES)), trace=trace
        )
    _cache["last_res"] = res
    results = res.results

    s_low = np.empty((B, K, 128), np.float32)
    s_recon = np.empty((B, V, 128), np.float32)
    for c in range(NCORES):
        r = results[c]
        s_low[:, 64 * c : 64 * (c + 1), :] = np.roll(r["slow_o"], 8 * c, axis=0)
        s_recon[:, 256 * c : 256 * (c + 1), :] = np.roll(r["srec_o"], 8 * c, axis=0)
    s_recon[:, 2048:2052, :] = results[0]["srem_o"]
    a_aug = results[0]["aaug_o"]
    return s_low, s_recon, a_aug


# revision 15
# speedup vs baseline: 1.0942x; 1.0851x over previous
# Trainium2 Bass kernel for nn_DSG_STGCN (PLV adjacency + Gumbel graph aug +
# lead-field/DCT projection). Self-contained: hardcodes shapes/sharding.
#
# Math (what the reference actually returns — the 2x GCN + GRU are dead code):
#   s_low[b]   = dct_m @ lead @ z[b].T            -> reassociated:  W_low.T @ zT
#   s_recon[b] = dct_m.T @ s_low[b]               -> (L.T @ G_slice).T @ zT,
#                with G = dct_m.T @ dct_m (input-independent constant)
#   a_aug      = sigmoid((log(e)-log(1-e)+g)/tau), e = .5*p + .5*a,
#                a from PLV threshold (Hilbert phases -> unit phasors -> grams),
#                p = sigmoid(hg @ hg.T), hg = relu((a @ mean_b z) @ w_gae + b)
#
# Sharding (8 cores): voxel-slice (256 rows each) of s_recon over all 64
# batches; k-slice (64 rows) of s_low; PLV grams data-parallel over batch
# (8 batches/core, realized by rotating z per core so the SPMD program is
# identical) with one small AllReduce of [R | P | sum_b z].
# Host ships z pre-transposed to [t, (b e)] — pure layout change that makes
# the z DMA contiguous and removes 64 on-chip transposes.
import os
import sys
import threading

import numpy as np

sys.path.insert(0, "/opt/trn_rl_repo")

NUM_ELEC = 128
T = 128
V = 2052
H = 64
K = 512
B = 64
VP = 2176  # V padded to 17*128
NCORES = 8
THRESH2 = float((0.5 * B * T) ** 2)  # plv>=0.5 on |.|^2 * (B*T)^2 scale

# float32r = reduced-precision fp32 matmul mode, 4x faster at N>=256.
USE_F32R = os.environ.get("KERNEL_F32", "0") != "1"

_lock = threading.Lock()
_cache = {}


def _dct_matrix_f64(N, Kd):
    n = np.arange(N)[None, :]
    k = np.arange(Kd)[:, None]
    m = np.sqrt(2.0 / N) * np.cos(np.pi * (2 * n + 1) * k / (2 * N))
    m[0, :] = 1.0 / np.sqrt(N)
    return m


def _consts():
    if "consts" in _cache:
        return _cache["consts"]
    dct = _dct_matrix_f64(V, K)  # [K, V]
    G = (dct.T @ dct).astype(np.float32)  # [V, V]
    dctT = dct.T.astype(np.float32)  # [V, K]
    idx = np.arange(T)
    hf = np.where(idx == 0, 1.0, np.where(idx < T // 2, 2.0, np.where(idx == T // 2, 1.0, 0.0)))
    A = np.fft.ifft(hf[:, None] * np.fft.fft(np.eye(T), axis=0), axis=0)
    Hm = np.imag(A).astype(np.float32)  # [T, T]; Re(analytic) == z
    ident = np.eye(128, dtype=np.float32)
    _cache["consts"] = (G, dctT, Hm, ident)
    return _cache["consts"]


def _build_nc():
    if "nc" in _cache:
        return _cache["nc"]
    import concourse.bacc as bacc
    import concourse.mybir as mybir
    import concourse.tile as tile
    from concourse.mybir import ActivationFunctionType as AF

    f32 = mybir.dt.float32
    f32r = mybir.dt.float32r
    fmm = f32r if USE_F32R else f32

    nc = bacc.Bacc(
        "TRN2",
        target_bir_lowering=False,
        debug=False,
        num_devices=NCORES,
    )

    # z pre-transposed on host: zt[t, b*128+e] = z_rot[b, e, t]
    zt_in = nc.dram_tensor("zt", [128, B * 128], fmm, kind="ExternalInput")
    lead_in = nc.dram_tensor("leadp", [VP, 128], fmm, kind="ExternalInput")
    gsl_in = nc.dram_tensor("gslp", [VP, 256], fmm, kind="ExternalInput")
    wk_in = nc.dram_tensor("wkp", [VP, 68], fmm, kind="ExternalInput")
    # packed constants: cr = [ident | hmT] (f32r), cf = [identf | wgae | gum | bgae]
    cr_in = nc.dram_tensor("cr", [128, 256], fmm, kind="ExternalInput")
    cf_in = nc.dram_tensor("cf", [128, 128 + H + 128 + 1], f32, kind="ExternalInput")

    slow_o = nc.dram_tensor("slow_o", [B, 64, 128], f32, kind="ExternalOutput")
    srec_o = nc.dram_tensor("srec_o", [B, 256, 128], f32, kind="ExternalOutput")
    srem_o = nc.dram_tensor("srem_o", [B, 4, 128], f32, kind="ExternalOutput")
    aaug_o = nc.dram_tensor("aaug_o", [128, 128], f32, kind="ExternalOutput")
    DEBUG = os.environ.get("KERNEL_DEBUG", "0") == "1"
    if DEBUG:
        pre_o = nc.dram_tensor("pre_o", [128, 384], f32, kind="ExternalOutput")
        post_o = nc.dram_tensor("post_o", [128, 384], f32, kind="ExternalOutput")

    with tile.TileContext(nc) as tc:
        with (
            tc.tile_pool(name="cpool", bufs=1) as cpool,
            tc.tile_pool(name="tpool", bufs=2) as tpool,
            tc.tile_pool(name="stpool", bufs=3) as stpool,
            tc.tile_pool(name="psum", bufs=1, space="PSUM") as psum,
            tc.tile_pool(name="dram", bufs=1, space="DRAM") as dram,
        ):
            # ---- constants + first z chunk ----
            cr_sb = cpool.tile([128, 256], fmm)
            nc.sync.dma_start(cr_sb[:], cr_in[:])
            id_sb = cr_sb[:, 0:128]
            hm_sb = cr_sb[:, 128:256]

            zT_sb = cpool.tile([128, B * 128], fmm)
            nc.sync.dma_start(zT_sb[:, 0:1024], zt_in[:, 0:1024])

            cf_sb = cpool.tile([128, 128 + H + 128 + 1], f32)
            nc.sync.dma_start(cf_sb[:], cf_in[:])
            idf_sb = cf_sb[:, 0:128]
            wgae_sb = cf_sb[:, 128 : 128 + H]
            gum_sb = cf_sb[:, 128 + H : 128 + H + 128]
            bgae_ap = cf_sb[0:H, 320:321]

            # ---- PLV chain on this core's 8 batches (zT cols 0:1024) ----
            xT_acc = cpool.tile([128, 128], f32)
            nc.gpsimd.tensor_copy(xT_acc[:], zT_sb[:, 0:128].bitcast(f32))
            for bl in range(1, 8):
                nc.gpsimd.tensor_add(
                    xT_acc[:], xT_acc[:], zT_sb[:, 128 * bl : 128 * (bl + 1)].bitcast(f32)
                )

            C_sb = cpool.tile([128, 1024], f32)
            S_sb = cpool.tile([128, 1024], f32)
            for h in range(2):
                rer = zT_sb[:, 512 * h : 512 * (h + 1)]
                re = rer.bitcast(f32)
                hb = psum.tile([128, 512], f32, tag="mm", bufs=3)
                nc.tensor.matmul(hb[:], hm_sb, rer)
                imsq = tpool.tile([128, 512], f32, tag="tmpa")
                nc.scalar.activation(imsq[:], hb[:], AF.Square)
                resq = tpool.tile([128, 512], f32, tag="tmpb")
                nc.gpsimd.tensor_mul(resq[:], re, re)
                mag2 = tpool.tile([128, 512], f32, tag="tmpc")
                nc.vector.tensor_add(mag2[:], imsq[:], resq[:])
                rinv = tpool.tile([128, 512], f32, tag="tmpe")
                nc.scalar.activation(rinv[:], mag2[:], AF.Abs_reciprocal_sqrt)
                nc.gpsimd.tensor_mul(C_sb[:, 512 * h : 512 * (h + 1)], re, rinv[:])
                nc.vector.tensor_mul(S_sb[:, 512 * h : 512 * (h + 1)], hb[:], rinv[:])

            r_ps = psum.tile([128, 128], f32, tag="r")
            p_ps = psum.tile([128, 128], f32, tag="p")
            for b in range(8):
                cb = C_sb[:, 128 * b : 128 * (b + 1)]
                nc.tensor.matmul(r_ps[:], cb, cb, start=(b == 0), stop=False)
            for b in range(8):
                sb_ = S_sb[:, 128 * b : 128 * (b + 1)]
                nc.tensor.matmul(r_ps[:], sb_, sb_, start=False, stop=(b == 7))
            for b in range(8):
                nc.tensor.matmul(
                    p_ps[:],
                    S_sb[:, 128 * b : 128 * (b + 1)],
                    C_sb[:, 128 * b : 128 * (b + 1)],
                    start=(b == 0),
                    stop=(b == 7),
                )

            # x (in [e,t] layout) = transpose(xT)
            x_ps = psum.tile([128, 128], f32, tag="acc")
            nc.tensor.transpose(x_ps[:], xT_acc[:], idf_sb)

            # ---- AllReduce [R | P | x], issued as early as possible ----
            ccin_sb = cpool.tile([128, 384], f32)
            nc.vector.tensor_copy(ccin_sb[:, 0:128], r_ps[:])
            nc.vector.tensor_copy(ccin_sb[:, 128:256], p_ps[:])
            nc.vector.tensor_copy(ccin_sb[:, 256:384], x_ps[:])
            cc_in = dram.tile([128, 384], f32)
            cc_out = dram.tile([128, 384], f32, addr_space="Shared")
            nc.gpsimd.dma_start(cc_in[:], ccin_sb[:])
            nc.gpsimd.collective_compute(
                "AllReduce",
                mybir.AluOpType.add,
                replica_groups=[list(range(NCORES))],
                ins=[cc_in.opt()],
                outs=[cc_out.opt()],
            )
            ccout_sb = cpool.tile([128, 384], f32)
            nc.gpsimd.dma_start(ccout_sb[:], cc_out[:])
            if DEBUG:
                nc.sync.dma_start(pre_o[:], ccin_sb[:])
                nc.sync.dma_start(post_o[:], ccout_sb[:])

            # ---- weight loads + precompute ----
            lead_sb = cpool.tile([128, 17 * 128], fmm)
            nc.sync.dma_start(
                lead_sb[:].rearrange("p (c t) -> p c t", c=17),
                lead_in.rearrange("(c p) t -> p c t", p=128),
            )
            gsl_sb = cpool.tile([128, 17 * 256], fmm)
            nc.sync.dma_start(
                gsl_sb[:].rearrange("p (c n) -> p c n", c=17),
                gsl_in.rearrange("(c p) n -> p c n", p=128),
            )
            wk_sb = cpool.tile([128, 17 * 68], fmm)
            nc.sync.dma_start(
                wk_sb[:].rearrange("p (c n) -> p c n", c=17),
                wk_in.rearrange("(c p) n -> p c n", p=128),
            )
            # remaining z
            nc.sync.dma_start(zT_sb[:, 1024:8192], zt_in[:, 1024:8192])

            m2t_ps = psum.tile([128, 256], f32, tag="wc")
            wc_ps = psum.tile([128, 68], f32, tag="p2")
            for k in range(17):
                lk = lead_sb[:, 128 * k : 128 * (k + 1)]
                nc.tensor.matmul(
                    m2t_ps[:],
                    lk,
                    gsl_sb[:, 256 * k : 256 * (k + 1)],
                    start=(k == 0),
                    stop=(k == 16),
                )
            for k in range(17):
                lk = lead_sb[:, 128 * k : 128 * (k + 1)]
                nc.tensor.matmul(
                    wc_ps[:],
                    lk,
                    wk_sb[:, 68 * k : 68 * (k + 1)],
                    start=(k == 0),
                    stop=(k == 16),
                )
            m2t_sb = cpool.tile([128, 256], fmm)
            nc.vector.tensor_copy(m2t_sb[:], m2t_ps[:])
            wc_sb = cpool.tile([128, 68], fmm)
            nc.vector.tensor_copy(wc_sb[:], wc_ps[:])

            # ---- streamed mains: per 8-batch group ----
            for g in range(8):
                zt_g = zT_sb[:, 1024 * g : 1024 * (g + 1)]
                for v in range(2):
                    st = stpool.tile([128, 1024], f32, tag="st", name=f"st{g}_{v}")
                    for j in range(2):
                        mm = psum.tile([128, 512], f32, tag="mm", bufs=3)
                        nc.tensor.matmul(
                            mm[:],
                            m2t_sb[:, 128 * v : 128 * (v + 1)],
                            zt_g[:, 512 * j : 512 * (j + 1)],
                        )
                        dst = st[:, 512 * j : 512 * (j + 1)]
                        if v == 0:
                            nc.vector.tensor_copy(dst, mm[:])
                        else:
                            nc.scalar.activation(dst, mm[:], AF.Copy)
                    nc.sync.dma_start(
                        srec_o[8 * g : 8 * (g + 1), 128 * v : 128 * (v + 1), :].rearrange(
                            "b v e -> v b e"
                        ),
                        st[:].rearrange("v (b e) -> v b e", b=8),
                    )
                cst = stpool.tile([68, 1024], f32, tag="cst", bufs=2, name=f"cst{g}")
                for j in range(2):
                    cm = psum.tile([68, 512], f32, tag="mm", bufs=3)
                    nc.tensor.matmul(cm[:], wc_sb[:], zt_g[:, 512 * j : 512 * (j + 1)])
                    dst = cst[:, 512 * j : 512 * (j + 1)]
                    if g % 2 == 0:
                        nc.vector.tensor_copy(dst, cm[:])
                    else:
                        nc.scalar.activation(dst, cm[:], AF.Copy)
                nc.sync.dma_start(
                    slow_o[8 * g : 8 * (g + 1), :, :].rearrange("b k e -> k b e"),
                    cst[0:64, :].rearrange("k (b e) -> k b e", b=8),
                )
                nc.sync.dma_start(
                    srem_o[8 * g : 8 * (g + 1), :, :].rearrange("b r e -> r b e"),
                    cst[64:68, :].rearrange("r (b e) -> r b e", b=8),
                )

            # ---- epilogue: a_aug (identical on every core) ----
            Rf = ccout_sb[:, 0:128]
            Pf = ccout_sb[:, 128:256]
            xs = ccout_sb[:, 256:384]
            # gumbel branch is input-only; scheduler hoists it early
            lu_sb = cpool.tile([128, 128], f32)
            nc.scalar.activation(lu_sb[:], gum_sb, AF.Ln)
            lv_sb = cpool.tile([128, 128], f32)
            nc.scalar.activation(lv_sb[:], lu_sb[:], AF.Ln, scale=-1.0)

            x_sb = cpool.tile([128, 128], f32)
            nc.vector.tensor_scalar_mul(x_sb[:], xs, 1.0 / B)
            pt_ps = psum.tile([128, 128], f32, tag="p")
            nc.tensor.transpose(pt_ps[:], Pf, idf_sb)
            i_sb = cpool.tile([128, 128], f32)
            nc.vector.tensor_sub(i_sb[:], Pf, pt_ps[:])
            i2_sb = cpool.tile([128, 128], f32)
            nc.vector.tensor_mul(i2_sb[:], i_sb[:], i_sb[:])
            r2_sb = cpool.tile([128, 128], f32)
            nc.vector.tensor_mul(r2_sb[:], Rf, Rf)
            m2_sb = cpool.tile([128, 128], f32)
            nc.vector.tensor_add(m2_sb[:], i2_sb[:], r2_sb[:])
            a_sb = cpool.tile([128, 128], f32)
            nc.vector.tensor_scalar(
                a_sb[:], m2_sb[:], THRESH2, None, op0=mybir.AluOpType.is_ge
            )
            axT_ps = psum.tile([128, 128], f32, tag="r")
            nc.tensor.matmul(axT_ps[:], x_sb[:], a_sb[:])
            axT_sb = cpool.tile([128, 128], f32)
            nc.vector.tensor_copy(axT_sb[:], axT_ps[:])
            hg_ps = psum.tile([H, 128], f32, tag="p")
            nc.tensor.matmul(hg_ps[:], wgae_sb, axT_sb[:])
            hg_sb = cpool.tile([H, 128], f32)
            nc.scalar.activation(hg_sb[:], hg_ps[:], AF.Relu, bias=bgae_ap)
            pp_ps = psum.tile([128, 128], f32, tag="r")
            nc.tensor.matmul(pp_ps[:], hg_sb[:], hg_sb[:])
            p_sb = cpool.tile([128, 128], f32)
            nc.scalar.activation(p_sb[:], pp_ps[:], AF.Sigmoid)
            epre_sb = cpool.tile([128, 128], f32)
            nc.vector.tensor_add(epre_sb[:], p_sb[:], a_sb[:])
            le_sb = cpool.tile([128, 128], f32)
            nc.scalar.activation(le_sb[:], epre_sb[:], AF.Ln, scale=0.5)
            l1me_sb = cpool.tile([128, 128], f32)
            nc.scalar.activation(l1me_sb[:], epre_sb[:], AF.Ln, bias=1.0, scale=-0.5)
            d1_sb = cpool.tile([128, 128], f32)
            nc.vector.tensor_sub(d1_sb[:], le_sb[:], l1me_sb[:])
            d2_sb = cpool.tile([128, 128], f32)
            nc.vector.tensor_sub(d2_sb[:], d1_sb[:], lv_sb[:])
            aaug_sb = cpool.tile([128, 128], f32)
            nc.scalar.activation(aaug_sb[:], d2_sb[:], AF.Sigmoid, scale=10.0)
            nc.sync.dma_start(aaug_o[:], aaug_sb[:])

    nc.compile()
    _cache["nc"] = nc
    return nc


def kernel(z, lead_field, gumbel_u, w_gae, b_gae, **_unused):
    from concourse.bass_utils import run_bass_kernel_spmd

    z = np.ascontiguousarray(np.asarray(z, dtype=np.float32))
    L = np.asarray(lead_field, dtype=np.float32)
    u = np.ascontiguousarray(np.asarray(gumbel_u, dtype=np.float32))
    w_gae = np.ascontiguousarray(np.asarray(w_gae, dtype=np.float32))
    b_gae = np.asarray(b_gae, dtype=np.float32).reshape(H)

    G, dctT, Hm, ident = _consts()
    Lp = np.zeros((VP, 128), np.float32)
    Lp[:V] = L
    cr = np.ascontiguousarray(
        np.concatenate([ident, np.ascontiguousarray(Hm.T)], axis=1)
    )
    cf = np.zeros((128, 128 + H + 128 + 1), np.float32)
    cf[:, 0:128] = ident
    cf[:, 128 : 128 + H] = w_gae
    cf[:, 128 + H : 128 + H + 128] = u
    cf[:H, 320] = b_gae

    nc = _build_nc()

    in_maps = []
    for c in range(NCORES):
        gsl = np.zeros((VP, 256), np.float32)
        gsl[:V] = G[:, 256 * c : 256 * (c + 1)]
        wk = np.zeros((VP, 68), np.float32)
        wk[:V, :64] = dctT[:, 64 * c : 64 * (c + 1)]
        wk[:V, 64:] = G[:, 2048:2052]
        zr = np.roll(z, -8 * c, axis=0)
        zt = np.ascontiguousarray(zr.reshape(B * 128, T).T)
        in_maps.append(
            {
                "zt": zt,
                "leadp": Lp,
                "gslp": gsl,
                "wkp": wk,
                "cr": cr,
                "cf": cf,
            }
        )

    trace = os.environ.get("KERNEL_TRACE", "0") == "1"
    with _lock:
        res = run_bass_kernel_spmd(
            nc, in_maps, core_ids=list(range(NCORES)), trace=trace
        )
    _cache["last_res"] = res
    results = res.results

    s_low = np.empty((B, K, 128), np.float32)
    s_recon = np.empty((B, V, 128), np.float32)
    for c in range(NCORES):
        r = results[c]
        s_low[:, 64 * c : 64 * (c + 1), :] = np.roll(r["slow_o"], 8 * c, axis=0)
        s_recon[:, 256 * c : 256 * (c + 1), :] = np.roll(r["srec_o"], 8 * c, axis=0)
    s_recon[:, 2048:2052, :] = results[0]["srem_o"]
    a_aug = results[0]["aaug_o"]
    return s_low, s_recon, a_aug


# revision 17
# speedup vs baseline: 1.1311x; 1.0338x over previous
# Trainium2 Bass kernel for nn_DSG_STGCN (PLV adjacency + Gumbel graph aug +
# lead-field/DCT projection). Self-contained: hardcodes shapes/sharding.
#
# Math (what the reference actually returns — the 2x GCN + GRU are dead code):
#   s_low[b]   = dct_m @ lead @ z[b].T            -> reassociated:  W_low.T @ zT
#   s_recon[b] = dct_m.T @ s_low[b]               -> (L.T @ G_slice).T @ zT,
#                with G = dct_m.T @ dct_m (input-independent constant)
#   a_aug      = sigmoid((log(e)-log(1-e)+g)/tau), e = .5*p + .5*a,
#                a from PLV threshold (Hilbert phases -> unit phasors -> grams),
#                p = sigmoid(hg @ hg.T), hg = relu((a @ mean_b z) @ w_gae + b)
#
# Sharding (8 cores): voxel-slice (256 rows each) of s_recon over all 64
# batches; k-slice (64 rows) of s_low; PLV grams data-parallel over batch
# (8 batches/core, realized by rotating z per core so the SPMD program is
# identical) with one small AllReduce of [R | P | sum_b z].
# Host ships z pre-transposed to [t, (b e)] — pure layout change that makes
# the z DMA contiguous and removes 64 on-chip transposes.
import os
import sys
import threading

import numpy as np

sys.path.insert(0, "/opt/trn_rl_repo")

NUM_ELEC = 128
T = 128
V = 2052
H = 64
K = 512
B = 64
VP = 2176  # V padded to 17*128
NCORES = 8
THRESH2 = float((0.5 * B * T) ** 2)  # plv>=0.5 on |.|^2 * (B*T)^2 scale

# float32r = reduced-precision fp32 matmul mode, 4x faster at N>=256.
USE_F32R = os.environ.get("KERNEL_F32", "0") != "1"

_lock = threading.Lock()
_cache = {}


def _dct_matrix_f64(N, Kd):
    n = np.arange(N)[None, :]
    k = np.arange(Kd)[:, None]
    m = np.sqrt(2.0 / N) * np.cos(np.pi * (2 * n + 1) * k / (2 * N))
    m[0, :] = 1.0 / np.sqrt(N)
    return m


def _consts():
    if "consts" in _cache:
        return _cache["consts"]
    dct = _dct_matrix_f64(V, K)  # [K, V]
    G = (dct.T @ dct).astype(np.float32)  # [V, V]
    dctT = dct.T.astype(np.float32)  # [V, K]
    idx = np.arange(T)
    hf = np.where(idx == 0, 1.0, np.where(idx < T // 2, 2.0, np.where(idx == T // 2, 1.0, 0.0)))
    A = np.fft.ifft(hf[:, None] * np.fft.fft(np.eye(T), axis=0), axis=0)
    Hm = np.imag(A).astype(np.float32)  # [T, T]; Re(analytic) == z
    ident = np.eye(128, dtype=np.float32)
    _cache["consts"] = (G, dctT, Hm, ident)
    return _cache["consts"]


def _build_nc():
    if "nc" in _cache:
        return _cache["nc"]
    import concourse.bacc as bacc
    import concourse.mybir as mybir
    import concourse.tile as tile
    from concourse.mybir import ActivationFunctionType as AF

    f32 = mybir.dt.float32
    f32r = mybir.dt.float32r
    fmm = f32r if USE_F32R else f32

    nc = bacc.Bacc(
        "TRN2",
        target_bir_lowering=False,
        debug=False,
        num_devices=NCORES,
    )

    # z pre-transposed on host: zt[t, b*128+e] = z_rot[b, e, t]
    zt_in = nc.dram_tensor("zt", [128, B * 128], fmm, kind="ExternalInput")
    lead_in = nc.dram_tensor("leadp", [VP, 128], fmm, kind="ExternalInput")
    gsl_in = nc.dram_tensor("gslp", [VP, 256], fmm, kind="ExternalInput")
    wk_in = nc.dram_tensor("wkp", [VP, 68], fmm, kind="ExternalInput")
    # packed constants: cr = [ident | hmT] (f32r), cf = [identf | wgae | gum | bgae]
    cr_in = nc.dram_tensor("cr", [128, 256], fmm, kind="ExternalInput")
    cf_in = nc.dram_tensor("cf", [128, 128 + H + 128 + 1], f32, kind="ExternalInput")

    srec_o = nc.dram_tensor("srec_o", [256, B, 128], f32, kind="ExternalOutput")
    combo_o = nc.dram_tensor("combo_o", [68, B, 128], f32, kind="ExternalOutput")
    aaug_o = nc.dram_tensor("aaug_o", [128, 128], f32, kind="ExternalOutput")
    DEBUG = os.environ.get("KERNEL_DEBUG", "0") == "1"
    if DEBUG:
        pre_o = nc.dram_tensor("pre_o", [128, 384], f32, kind="ExternalOutput")
        post_o = nc.dram_tensor("post_o", [128, 384], f32, kind="ExternalOutput")

    with tile.TileContext(nc) as tc:
        with (
            tc.tile_pool(name="cpool", bufs=1) as cpool,
            tc.tile_pool(name="tpool", bufs=2) as tpool,
            tc.tile_pool(name="stpool", bufs=3) as stpool,
            tc.tile_pool(name="psum", bufs=1, space="PSUM") as psum,
            tc.tile_pool(name="dram", bufs=1, space="DRAM") as dram,
        ):
            # ---- constants + first z chunk ----
            cr_sb = cpool.tile([128, 256], fmm)
            nc.sync.dma_start(cr_sb[:], cr_in[:])
            id_sb = cr_sb[:, 0:128]
            hm_sb = cr_sb[:, 128:256]

            zT_sb = cpool.tile([128, B * 128], fmm)
            nc.sync.dma_start(zT_sb[:, 0:1024], zt_in[:, 0:1024])

            cf_sb = cpool.tile([128, 128 + H + 128 + 1], f32)
            nc.sync.dma_start(cf_sb[:], cf_in[:])
            idf_sb = cf_sb[:, 0:128]
            wgae_sb = cf_sb[:, 128 : 128 + H]
            gum_sb = cf_sb[:, 128 + H : 128 + H + 128]
            bgae_ap = cf_sb[0:H, 320:321]

            # ---- PLV chain on this core's 8 batches (zT cols 0:1024) ----
            xT_acc = cpool.tile([128, 128], f32)
            nc.vector.tensor_copy(xT_acc[:], zT_sb[:, 0:128].bitcast(f32))
            for bl in range(1, 8):
                nc.vector.tensor_add(
                    xT_acc[:], xT_acc[:], zT_sb[:, 128 * bl : 128 * (bl + 1)].bitcast(f32)
                )

            C_sb = cpool.tile([128, 1024], f32)
            S_sb = cpool.tile([128, 1024], f32)
            for h in range(2):
                rer = zT_sb[:, 512 * h : 512 * (h + 1)]
                re = rer.bitcast(f32)
                hb = psum.tile([128, 512], f32, tag="mm", bufs=3)
                nc.tensor.matmul(hb[:], hm_sb, rer)
                imsq = tpool.tile([128, 512], f32, tag="tmpa")
                nc.scalar.activation(imsq[:], hb[:], AF.Square)
                resq = tpool.tile([128, 512], f32, tag="tmpb")
                nc.vector.tensor_mul(resq[:], re, re)
                mag2 = tpool.tile([128, 512], f32, tag="tmpc")
                nc.vector.tensor_add(mag2[:], imsq[:], resq[:])
                rinv = tpool.tile([128, 512], f32, tag="tmpe")
                nc.scalar.activation(rinv[:], mag2[:], AF.Abs_reciprocal_sqrt)
                nc.vector.tensor_mul(C_sb[:, 512 * h : 512 * (h + 1)], re, rinv[:])
                nc.vector.tensor_mul(S_sb[:, 512 * h : 512 * (h + 1)], hb[:], rinv[:])

            r_ps = psum.tile([128, 128], f32, tag="r")
            p_ps = psum.tile([128, 128], f32, tag="p")
            for b in range(8):
                cb = C_sb[:, 128 * b : 128 * (b + 1)]
                nc.tensor.matmul(r_ps[:], cb, cb, start=(b == 0), stop=False)
            for b in range(8):
                sb_ = S_sb[:, 128 * b : 128 * (b + 1)]
                nc.tensor.matmul(r_ps[:], sb_, sb_, start=False, stop=(b == 7))
            for b in range(8):
                nc.tensor.matmul(
                    p_ps[:],
                    S_sb[:, 128 * b : 128 * (b + 1)],
                    C_sb[:, 128 * b : 128 * (b + 1)],
                    start=(b == 0),
                    stop=(b == 7),
                )

            # x (in [e,t] layout) = transpose(xT)
            x_ps = psum.tile([128, 128], f32, tag="acc")
            nc.tensor.transpose(x_ps[:], xT_acc[:], idf_sb)

            # ---- AllReduce [R | P | x], issued as early as possible ----
            ccin_sb = cpool.tile([128, 384], f32)
            nc.vector.tensor_copy(ccin_sb[:, 0:128], r_ps[:])
            nc.vector.tensor_copy(ccin_sb[:, 128:256], p_ps[:])
            nc.vector.tensor_copy(ccin_sb[:, 256:384], x_ps[:])
            cc_in = dram.tile([128, 384], f32)
            cc_out = dram.tile([128, 384], f32, addr_space="Shared")
            nc.gpsimd.dma_start(cc_in[:], ccin_sb[:])
            nc.gpsimd.collective_compute(
                "AllReduce",
                mybir.AluOpType.add,
                replica_groups=[list(range(NCORES))],
                ins=[cc_in.opt()],
                outs=[cc_out.opt()],
            )
            ccout_sb = cpool.tile([128, 384], f32)
            nc.gpsimd.dma_start(ccout_sb[:], cc_out[:])
            if DEBUG:
                nc.sync.dma_start(pre_o[:], ccin_sb[:])
                nc.sync.dma_start(post_o[:], ccout_sb[:])

            # ---- weight loads + precompute ----
            lead_sb = cpool.tile([128, 17 * 128], fmm)
            nc.sync.dma_start(
                lead_sb[:].rearrange("p (c t) -> p c t", c=17),
                lead_in.rearrange("(c p) t -> p c t", p=128),
            )
            gsl_sb = cpool.tile([128, 17 * 256], fmm)
            nc.sync.dma_start(
                gsl_sb[:].rearrange("p (c n) -> p c n", c=17),
                gsl_in.rearrange("(c p) n -> p c n", p=128),
            )
            wk_sb = cpool.tile([128, 17 * 68], fmm)
            nc.sync.dma_start(
                wk_sb[:].rearrange("p (c n) -> p c n", c=17),
                wk_in.rearrange("(c p) n -> p c n", p=128),
            )
            # remaining z
            nc.sync.dma_start(zT_sb[:, 1024:8192], zt_in[:, 1024:8192])

            m2t_ps = psum.tile([128, 256], f32, tag="wc")
            wc_ps = psum.tile([128, 68], f32, tag="p2")
            for k in range(17):
                lk = lead_sb[:, 128 * k : 128 * (k + 1)]
                nc.tensor.matmul(
                    m2t_ps[:],
                    lk,
                    gsl_sb[:, 256 * k : 256 * (k + 1)],
                    start=(k == 0),
                    stop=(k == 16),
                )
            for k in range(17):
                lk = lead_sb[:, 128 * k : 128 * (k + 1)]
                nc.tensor.matmul(
                    wc_ps[:],
                    lk,
                    wk_sb[:, 68 * k : 68 * (k + 1)],
                    start=(k == 0),
                    stop=(k == 16),
                )
            m2t_sb = cpool.tile([128, 256], fmm)
            nc.vector.tensor_copy(m2t_sb[:], m2t_ps[:])
            wc_sb = cpool.tile([128, 68], fmm)
            nc.vector.tensor_copy(wc_sb[:], wc_ps[:])

            # ---- streamed mains: per 8-batch group ----
            for g in range(8):
                zt_g = zT_sb[:, 1024 * g : 1024 * (g + 1)]
                for v in range(2):
                    st = stpool.tile([128, 1024], f32, tag="st", name=f"st{g}_{v}")
                    for j in range(2):
                        mm = psum.tile([128, 512], f32, tag="mm", bufs=3)
                        nc.tensor.matmul(
                            mm[:],
                            m2t_sb[:, 128 * v : 128 * (v + 1)],
                            zt_g[:, 512 * j : 512 * (j + 1)],
                        )
                        dst = st[:, 512 * j : 512 * (j + 1)]
                        if v == 0:
                            nc.vector.tensor_copy(dst, mm[:])
                        else:
                            nc.scalar.activation(dst, mm[:], AF.Copy)
                    nc.sync.dma_start(
                        srec_o[128 * v : 128 * (v + 1), 8 * g : 8 * (g + 1), :],
                        st[:].rearrange("v (b e) -> v b e", b=8),
                    )
                cst = stpool.tile([68, 1024], f32, tag="cst", bufs=2, name=f"cst{g}")
                for j in range(2):
                    cm = psum.tile([68, 512], f32, tag="mm", bufs=3)
                    nc.tensor.matmul(cm[:], wc_sb[:], zt_g[:, 512 * j : 512 * (j + 1)])
                    dst = cst[:, 512 * j : 512 * (j + 1)]
                    if g % 2 == 0:
                        nc.vector.tensor_copy(dst, cm[:])
                    else:
                        nc.scalar.activation(dst, cm[:], AF.Copy)
                nc.sync.dma_start(
                    combo_o[:, 8 * g : 8 * (g + 1), :],
                    cst[:].rearrange("k (b e) -> k b e", b=8),
                )

            # ---- epilogue: a_aug (identical on every core) ----
            Rf = ccout_sb[:, 0:128]
            Pf = ccout_sb[:, 128:256]
            xs = ccout_sb[:, 256:384]
            # gumbel branch is input-only; scheduler hoists it early
            lu_sb = cpool.tile([128, 128], f32)
            nc.scalar.activation(lu_sb[:], gum_sb, AF.Ln)
            lv_sb = cpool.tile([128, 128], f32)
            nc.scalar.activation(lv_sb[:], lu_sb[:], AF.Ln, scale=-1.0)

            pt_ps = psum.tile([128, 128], f32, tag="p")
            nc.tensor.transpose(pt_ps[:], Pf, idf_sb)
            i_sb = cpool.tile([128, 128], f32)
            nc.vector.tensor_sub(i_sb[:], Pf, pt_ps[:])
            i2_sb = cpool.tile([128, 128], f32)
            nc.vector.tensor_mul(i2_sb[:], i_sb[:], i_sb[:])
            r2_sb = cpool.tile([128, 128], f32)
            nc.vector.tensor_mul(r2_sb[:], Rf, Rf)
            m2_sb = cpool.tile([128, 128], f32)
            nc.vector.tensor_add(m2_sb[:], i2_sb[:], r2_sb[:])
            a_sb = cpool.tile([128, 128], f32)
            nc.vector.tensor_scalar(
                a_sb[:], m2_sb[:], THRESH2, None, op0=mybir.AluOpType.is_ge
            )
            axT_ps = psum.tile([128, 128], f32, tag="r")
            nc.tensor.matmul(axT_ps[:], xs, a_sb[:])
            axT_sb = cpool.tile([128, 128], f32)
            nc.vector.tensor_copy(axT_sb[:], axT_ps[:])
            hg_ps = psum.tile([H, 128], f32, tag="p")
            nc.tensor.matmul(hg_ps[:], wgae_sb, axT_sb[:])
            hg_sb = cpool.tile([H, 128], f32)
            nc.scalar.activation(hg_sb[:], hg_ps[:], AF.Relu, bias=bgae_ap, scale=1.0 / B)
            pp_ps = psum.tile([128, 128], f32, tag="r")
            nc.tensor.matmul(pp_ps[:], hg_sb[:], hg_sb[:])
            p_sb = cpool.tile([128, 128], f32)
            nc.scalar.activation(p_sb[:], pp_ps[:], AF.Sigmoid)
            epre_sb = cpool.tile([128, 128], f32)
            nc.vector.tensor_add(epre_sb[:], p_sb[:], a_sb[:])
            le_sb = cpool.tile([128, 128], f32)
            nc.scalar.activation(le_sb[:], epre_sb[:], AF.Ln, scale=0.5)
            l1me_sb = cpool.tile([128, 128], f32)
            nc.scalar.activation(l1me_sb[:], epre_sb[:], AF.Ln, bias=1.0, scale=-0.5)
            d1_sb = cpool.tile([128, 128], f32)
            nc.vector.tensor_sub(d1_sb[:], le_sb[:], l1me_sb[:])
            d2_sb = cpool.tile([128, 128], f32)
            nc.vector.tensor_sub(d2_sb[:], d1_sb[:], lv_sb[:])
            aaug_sb = cpool.tile([128, 128], f32)
            nc.scalar.activation(aaug_sb[:], d2_sb[:], AF.Sigmoid, scale=10.0)
            nc.gpsimd.dma_start(aaug_o[:], aaug_sb[:])

    nc.compile()
    _cache["nc"] = nc
    return nc


def kernel(z, lead_field, gumbel_u, w_gae, b_gae, **_unused):
    from concourse.bass_utils import run_bass_kernel_spmd

    z = np.ascontiguousarray(np.asarray(z, dtype=np.float32))
    L = np.asarray(lead_field, dtype=np.float32)
    u = np.ascontiguousarray(np.asarray(gumbel_u, dtype=np.float32))
    w_gae = np.ascontiguousarray(np.asarray(w_gae, dtype=np.float32))
    b_gae = np.asarray(b_gae, dtype=np.float32).reshape(H)

    G, dctT, Hm, ident = _consts()
    Lp = np.zeros((VP, 128), np.float32)
    Lp[:V] = L
    cr = np.ascontiguousarray(
        np.concatenate([ident, np.ascontiguousarray(Hm.T)], axis=1)
    )
    cf = np.zeros((128, 128 + H + 128 + 1), np.float32)
    cf[:, 0:128] = ident
    cf[:, 128 : 128 + H] = w_gae
    cf[:, 128 + H : 128 + H + 128] = u
    cf[:H, 320] = b_gae

    nc = _build_nc()

    in_maps = []
    for c in range(NCORES):
        gsl = np.zeros((VP, 256), np.float32)
        gsl[:V] = G[:, 256 * c : 256 * (c + 1)]
        wk = np.zeros((VP, 68), np.float32)
        wk[:V, :64] = dctT[:, 64 * c : 64 * (c + 1)]
        wk[:V, 64:] = G[:, 2048:2052]
        zr = np.roll(z, -8 * c, axis=0)
        zt = np.ascontiguousarray(zr.reshape(B * 128, T).T)
        in_maps.append(
            {
                "zt": zt,
                "leadp": Lp,
                "gslp": gsl,
                "wkp": wk,
                "cr": cr,
                "cf": cf,
            }
        )

    trace = os.environ.get("KERNEL_TRACE", "0") == "1"
    tc_env = os.environ.get("KERNEL_TRACE_CORES", "")
    kw = {}
    if tc_env:
        kw["trace_cores"] = [int(x) for x in tc_env.split(",")]
    with _lock:
        res = run_bass_kernel_spmd(
            nc, in_maps, core_ids=list(range(NCORES)), trace=trace, **kw
        )
    _cache["last_res"] = res
    results = res.results

    s_low = np.empty((B, K, 128), np.float32)
    s_recon = np.empty((B, V, 128), np.float32)
    for c in range(NCORES):
        r = results[c]
        combo = np.roll(r["combo_o"].transpose(1, 0, 2), 8 * c, axis=0)
        s_low[:, 64 * c : 64 * (c + 1), :] = combo[:, :64, :]
        s_recon[:, 256 * c : 256 * (c + 1), :] = np.roll(
            r["srec_o"].transpose(1, 0, 2), 8 * c, axis=0
        )
        if c == 0:
            s_recon[:, 2048:2052, :] = combo[:, 64:68, :]
    a_aug = results[0]["aaug_o"]
    return s_low, s_recon, a_aug
